# revision 39
# baseline (speedup 1.0000x reference)
"""EnhancedAttentionModule Trainium2 kernel.

x: [16, 512, 4096] f32.  Module:
    pooled = mean_n(x)                      # [B, C]
    h  = relu(pooled @ w1.T + b1)           # [B, C/4]
    ca = sigmoid(h @ w2.T + b2)             # [B, C]  (channel attention)
    x_ca = x * ca[:, :, None]
    h2 = BN(w3 @ x_ca + b3); h2 = relu(h2)  # [B, C/4, N]
    sa = sigmoid(w4 @ h2 + b4)              # [B, 1, N] (spatial attention)
    out = x + x_ca * sa = x * (1 + ca*sa)

Restructuring:
  - The problem is HBM-DMA bound: all DMA serializes on one shared
    engine pool at ~360 GB/s. x (and out) are stored in DRAM as fp16
    (host converts); accumulation stays f32 in PSUM. Measured
    end-to-end rel err ~1e-3 (gate 2e-2).
  - mean divisor folded into w1, BN folded into w3/bias (host); all
    matmul weights shipped fp16 in one blob (Matmult forbids mixing
    16/32-bit inputs; fp16 runs 1 cycle/row on PE).
  - ca folded into the w3 matmul weights on device (w3e = w3Ti * ca).
  - sa is produced REPLICATED across all 128 partitions for free: the
    w4 matmul uses a [CR, 128] all-equal-columns lhsT, so the sigmoid
    (cost = free size) directly yields [128, N] fp16 sa.
  - out = x * (1 + ca[c]*sa[n]): s2 = sa*ca_j + 1 via DVE tensor_scalar
    (4x fp16 mode); the multiplies are split DVE (2x fp16 mode) / Pool
    (gpsimd tensor_tensor) per 1024-block so neither engine's in-order
    queue becomes the tail.
  - pooled sums all run on DVE (in-place x*1.0 with accum_out, 4x
    mode); batch-1's are interleaved between batch-0's multiply groups
    so the in-order DVE queue never head-blocks on a not-yet-loaded
    tile.
  - stores go out in 1024-wide quarter-tiles immediately after each
    block's multiply, so the serial DMA queue never waits for a full
    tile; loads for both batches are issued up front.
  - a t~0 dummy sigmoid pins the one ACT table set that covers
    Copy/Relu/Sigmoid (no mid-chain 1.3us table switches); tiny dummy
    matmuls tied to each x-tile load keep the PE p-state ramped.

Sharding: data-parallel over batch. 8 cores x 2 batches each. Weights
replicated. No collectives. Per core: 8.4 MB HBM read + 8.4 MB write
plus ~0.5 MB weights - the serial-DMA roofline for this problem.
"""

import numpy as np

B, C, N = 16, 512, 4096
CR = C // 4  # 128
P = 128      # partitions
NCORES = 8
BPC = B // NCORES        # batches per core = 2
CCH = C // P             # channel chunks per batch = 4
NK = N // 1024           # 1024-wide chain blocks = 4
BN_EPS = 1e-5

# fp16 weight blob ([128, HBLOB])
_W3 = 0          # w3Ti as [p, j, m]: cols [0, 512)
_W1 = 512        # w1nT as [p, j, m]: cols [512, 1024)
_W2 = 1024       # w2T: cols [1024, 1536)
_W4 = 1536       # w4 replicated into 128 cols: [1536, 1664)
HBLOB = 1664
# f32 small blob ([128, FBLOB]): biases
_B1 = 0
_B3 = 1
_B2C = 2         # cols [2, 6)
_B4 = 6          # replicated down all 128 rows
FBLOB = 7

_CACHE = {}


def _build(n_iter=1):
    import concourse.bacc as bacc
    import concourse.tile as tile
    from concourse import mybir

    f32 = mybir.dt.float32
    f16 = mybir.dt.float16
    AF = mybir.ActivationFunctionType
    ALU = mybir.AluOpType

    nc = bacc.Bacc(None)

    xs = nc.dram_tensor("xs", [BPC * C, N], f16, kind="ExternalInput")
    out = nc.dram_tensor("outv", [BPC * C, N], f16, kind="ExternalOutput")
    wbh_d = nc.dram_tensor("wblobh", [P, HBLOB], f16, kind="ExternalInput")
    wbf_d = nc.dram_tensor("wblobf", [P, FBLOB], f32, kind="ExternalInput")

    xs_t = xs.rearrange("(t p) n -> t p n", p=P)      # 8 tiles [128, 4096]
    out_t = out.rearrange("(t p) n -> t p n", p=P)

    with tile.TileContext(nc) as tc:
        with (
            tc.tile_pool(name="wpool", bufs=1) as wpool,
            tc.tile_pool(name="xpool", bufs=BPC * CCH) as xpool,
            tc.tile_pool(name="opool", bufs=BPC * CCH * NK) as opool,
            tc.tile_pool(name="small", bufs=6) as small,
            tc.tile_pool(name="wefpool", bufs=2 * CCH) as wefpool,
            tc.tile_pool(name="h2spool", bufs=2) as h2spool,
            tc.tile_pool(name="sapool", bufs=2) as sapool,
            tc.tile_pool(name="s2pool", bufs=12) as s2pool,
            tc.tile_pool(name="ps_hca", bufs=1, space="PSUM") as ps_hca,
            tc.tile_pool(name="ps_h2", bufs=2, space="PSUM") as ps_h2,
            tc.tile_pool(name="ps_sa", bufs=1, space="PSUM") as ps_sa,
            tc.tile_pool(name="ps_junk", bufs=1, space="PSUM") as ps_junk,
        ):
            wbh = wpool.tile([P, HBLOB], f16)
            wbf = wpool.tile([P, FBLOB], f32)
            w3Ti_sb = wbh[:, _W3 : _W3 + 512].rearrange("p (j m) -> p j m", j=CCH)
            w1nT_sb = wbh[:, _W1 : _W1 + 512].rearrange("p (j m) -> p j m", j=CCH)
            w2T_sb = wbh[:, _W2 : _W2 + 512]
            w4r_sb = wbh[:, _W4 : _W4 + P]
            b1_sb = wbf[:, _B1 : _B1 + 1]
            b3e_sb = wbf[:, _B3 : _B3 + 1]
            b2c_sb = wbf[:, _B2C : _B2C + CCH]
            b4_sb = wbf[:, _B4 : _B4 + 1]

            # dummy tiles: pin the sigmoid act table at t~0 (the
            # sigmoid_and_others set also serves Copy and Relu, so no
            # further table loads occur) and seed the PE p-state ramp.
            junk = wpool.tile([P, 2], f16)
            junkf = wpool.tile([1, 2], f32)
            psj = ps_junk.tile([P, 2], f32)
            nc.vector.memset(junk, 1.0)
            nc.scalar.activation(junkf, junk[0:1, :], AF.Sigmoid)
            nc.tensor.matmul(psj[0:1, :], lhsT=junk[:, 0:1], rhs=junk, start=True, stop=True)

            def pe_warm(t):
                # tiny matmul tied to a fresh x tile: keeps the PE busy
                # streak alive through the load phase so the real h2
                # matmuls run at the full 2.4 GHz p-state.
                nc.tensor.matmul(
                    psj[0:1, 0:1], lhsT=t[:, 0:1], rhs=t[:, 1:2],
                    start=True, stop=True,
                )

            def emit_weight_dmas():
                nc.sync.dma_start(out=wbh, in_=wbh_d[:, :])
                nc.sync.dma_start(out=wbf, in_=wbf_d[:, :])

            for _it in range(n_iter):
                # ---- all x loads emitted up front (both batches) so the
                # serial DMA resource runs them back-to-back.
                xts = []
                for b in range(BPC):
                    xt = []
                    for j in range(CCH):
                        t = xpool.tile([P, N], f16, tag="xt")
                        xt.append(t)
                        if j == 3:
                            # last tile of the batch: split load so the
                            # critical pooled reduction starts on the
                            # first half while the second streams in.
                            nc.sync.dma_start(
                                out=t[:, 0:2048], in_=xs_t[b * CCH + j][:, 0:2048]
                            )
                            nc.sync.dma_start(
                                out=t[:, 2048:4096],
                                in_=xs_t[b * CCH + j][:, 2048:4096],
                            )
                        else:
                            nc.sync.dma_start(out=t, in_=xs_t[b * CCH + j])
                        pe_warm(t)
                    xts.append(xt)
                    if b == 0 and _it == 0:
                        emit_weight_dmas()

                def emit_pooled_halves(t, pooled, j):
                    # two half-tile reductions (the halves arrive as
                    # separate DMAs); the MLP matmul accumulates the
                    # partials directly - it is linear in pooled.
                    for h in range(2):
                        pj = small.tile([P, 1], f32, tag="pooled")
                        nc.vector.tensor_scalar(
                            t[:, h * 2048 : (h + 1) * 2048],
                            t[:, h * 2048 : (h + 1) * 2048],
                            1.0, 0.0, ALU.mult, ALU.add, accum_out=pj,
                        )
                        ph = small.tile([P, 1], f16, tag="pooledh")
                        nc.gpsimd.tensor_copy(ph, pj)
                        pooled.append((j, ph))
                    pe_warm(t)

                def emit_pooled(t, act=False):
                    # in-place identity with free-dim accumulator: ACT
                    # (copy) for tiles arriving while ACT idles, DVE (4x
                    # fp16 tensor_scalar) for critical late tiles. The
                    # tiny f32->fp16 copy for the fp16 MLP matmul runs on
                    # Pool to keep the DVE queue clear.
                    pj = small.tile([P, 1], f32, tag="pooled")
                    if act:
                        nc.scalar.activation(t, t, AF.Copy, accum_out=pj)
                    else:
                        nc.vector.tensor_scalar(
                            t, t, 1.0, 0.0, ALU.mult, ALU.add, accum_out=pj
                        )
                    ph = small.tile([P, 1], f16, tag="pooledh")
                    nc.gpsimd.tensor_copy(ph, pj)
                    pe_warm(t)
                    return ph

                def emit_mlp(pooled):
                    # channel attention MLP (all-fp16 matmuls); returns
                    # ca as per-partition columns [P, CCH] f32 (scalar
                    # ptr operands must be f32). `pooled` is a list of
                    # (j, partial-sum) pairs: a tile's pooled sum may
                    # arrive as several partials (the matmul accumulates
                    # them - it's linear).
                    psum_hca = ps_hca.tile([P, 8], f32, tag="hca")
                    psum_h = psum_hca[:, 0:1]
                    psum_ca = psum_hca[:, 4:8]
                    for i, (j, ph) in enumerate(pooled):
                        nc.tensor.matmul(
                            psum_h,
                            lhsT=w1nT_sb[:, j, :],
                            rhs=ph,
                            start=(i == 0),
                            stop=(i == len(pooled) - 1),
                        )
                    h_sb = small.tile([P, 1], f16, tag="h")
                    nc.scalar.activation(h_sb, psum_h, AF.Relu, bias=b1_sb)
                    for j in range(CCH):
                        nc.tensor.matmul(
                            psum_ca[:, j : j + 1],
                            lhsT=w2T_sb[:, j * P : (j + 1) * P],
                            rhs=h_sb,
                            start=True,
                            stop=True,
                        )
                    ca_sb = small.tile([P, CCH], f32, tag="ca")
                    for j in range(CCH):
                        nc.scalar.activation(
                            ca_sb[:, j : j + 1],
                            psum_ca[:, j : j + 1],
                            AF.Sigmoid,
                            bias=b2c_sb[:, j : j + 1],
                        )
                    # fold ca into w3 (fp16 weights for the fp16 h2 matmul)
                    w3e = []
                    for j in range(CCH):
                        we = wefpool.tile([P, CR], f16, tag="w3e")
                        nc.vector.tensor_scalar_mul(
                            we, w3Ti_sb[:, j, :], ca_sb[:, j : j + 1]
                        )
                        w3e.append(we)
                    pe_warm(w3e[0])
                    return ca_sb, w3e

                def emit_chain_block(xt, w3e, sa_sb, k):
                    # h2 = relu(w3e @ x + b3e); sa = sigmoid(w4r @ h2 + b4)
                    # on one 1024-wide block, sa replicated on all rows.
                    # matmul outputs are 512-wide (a PSUM bank holds 512
                    # f32); the ACT ops span both banks in one 1024-wide
                    # instruction (PSUM-crossing APs are legal for ACT).
                    lo = k * 1024
                    psum_h2 = ps_h2.tile([P, 1024], f32, tag="ph2")
                    for hh in range(2):
                        o = lo + hh * 512
                        for j in range(CCH):
                            nc.tensor.matmul(
                                psum_h2[:, hh * 512 : (hh + 1) * 512],
                                lhsT=w3e[j],
                                rhs=xt[j][:, o : o + 512],
                                start=(j == 0),
                                stop=(j == CCH - 1),
                            )
                    h2s = h2spool.tile([P, 1024], f16, tag="h2s")
                    nc.scalar.activation(h2s, psum_h2, AF.Relu, bias=b3e_sb)
                    psum_sa = ps_sa.tile([P, 1024], f32, tag="psa")
                    for hh in range(2):
                        nc.tensor.matmul(
                            psum_sa[:, hh * 512 : (hh + 1) * 512],
                            lhsT=w4r_sb,
                            rhs=h2s[:, hh * 512 : (hh + 1) * 512],
                            start=True,
                            stop=True,
                        )
                    nc.scalar.activation(
                        sa_sb[:, lo : lo + 1024], psum_sa, AF.Sigmoid, bias=b4_sb
                    )

                def emit_mul_group(b, xt, ot, ca_sb, sa_sb, k, s2_act=False, js=None):
                    # out = x * (1 + ca_j*sa) for 1024-block k, all 4 j.
                    # s2 on DVE (tensor_scalar, 4x); multiplies j0-j2 on
                    # DVE (2x), j3 on Pool (gpsimd) so neither in-order
                    # queue becomes the tail. Multiplies write SEPARATE
                    # output tiles: writing the x tile in place makes
                    # every mul wait (whole-tile WAR) for the tile's last
                    # h2-matmul read, which costs ~8us per batch. Each
                    # half-tile store goes out immediately after that
                    # tile's mul in an odd group (its 2 blocks are done).
                    lo = k * 1024
                    for j in js if js is not None else (0, 3, 1, 2):
                        s2 = s2pool.tile([P, 1024], f16, tag="s2")
                        if s2_act:
                            # ACT is idle after its sigmoid chains; its
                            # func(in*scale+bias) form computes sa*ca+1
                            # directly, relieving the DVE tail.
                            nc.scalar.activation(
                                s2,
                                sa_sb[:, lo : lo + 1024],
                                AF.Copy,
                                bias=1.0,
                                scale=ca_sb[:, j : j + 1],
                            )
                        else:
                            nc.vector.tensor_scalar(
                                s2,
                                sa_sb[:, lo : lo + 1024],
                                ca_sb[:, j : j + 1],
                                1.0,
                                ALU.mult,
                                ALU.add,
                            )
                        # Pool helps early/mid groups only: its 2.13us/mul
                        # pace must never gate the batch's last stores.
                        on_pool = (j == 3 and (k < 3 or b == 0)) or (j == 2 and b == 0 and k < 3)
                        eng = nc.gpsimd if on_pool else nc.vector
                        eng.tensor_mul(
                            ot[j][k],
                            xt[j][:, lo : lo + 1024],
                            s2,
                        )
                        for hh in range(2):
                            o = hh * 512
                            nc.sync.dma_start(
                                out=out_t[b * CCH + j][:, lo + o : lo + o + 512],
                                in_=ot[j][k][:, o : o + 512],
                            )

                # ---------- batch 0 ----------
                xt0, xt1 = xts
                # per-quarter output tiles: a shared per-tile output
                # buffer makes each store (reader) gate the next group's
                # mul (writer) via whole-tile WAR + sem prop. Dedicated
                # [P, 1024] tiles kill that false dependency.
                ot0 = [
                    [
                        opool.tile([P, 1024], f16, tag="ot", name=f"ot0_{j}_{k}_{_it}")
                        for k in range(NK)
                    ]
                    for j in range(CCH)
                ]
                ot1 = [
                    [
                        opool.tile([P, 1024], f16, tag="ot", name=f"ot1_{j}_{k}_{_it}")
                        for k in range(NK)
                    ]
                    for j in range(CCH)
                ]
                pooled0 = [(j, emit_pooled(xt0[j], act=(j < 2))) for j in range(3)]
                emit_pooled_halves(xt0[3], pooled0, 3)
                ca0, w3e0 = emit_mlp(pooled0)
                sa0 = sapool.tile([P, N], f16, tag="sa")
                pooled1 = []

                # chain blocks + mul groups pipelined; batch-1 pooled
                # interleaved into the DVE stream as its tiles land.
                emit_chain_block(xt0, w3e0, sa0, 0)
                pooled1.append((0, emit_pooled(xt1[0])))  # b1 t0 (early)
                emit_chain_block(xt0, w3e0, sa0, 1)
                emit_mul_group(0, xt0, ot0, ca0, sa0, 0)
                pooled1.append((1, emit_pooled(xt1[1])))
                emit_chain_block(xt0, w3e0, sa0, 2)
                emit_mul_group(0, xt0, ot0, ca0, sa0, 1)
                pooled1.append((2, emit_pooled(xt1[2])))
                emit_chain_block(xt0, w3e0, sa0, 3)
                emit_mul_group(0, xt0, ot0, ca0, sa0, 2, js=(0, 3))
                emit_pooled_halves(xt1[3], pooled1, 3)
                emit_mul_group(0, xt0, ot0, ca0, sa0, 2, js=(1, 2))

                # ---------- batch 1 (MLP emitted before batch 0's last
                # mul group so its DVE folds aren't queued behind it) ----
                ca1, w3e1 = emit_mlp(pooled1)
                sa1 = sapool.tile([P, N], f16, tag="sa")
                emit_mul_group(0, xt0, ot0, ca0, sa0, 3)
                emit_chain_block(xt1, w3e1, sa1, 0)
                emit_chain_block(xt1, w3e1, sa1, 1)
                emit_mul_group(1, xt1, ot1, ca1, sa1, 0)
                emit_chain_block(xt1, w3e1, sa1, 2)
                emit_mul_group(1, xt1, ot1, ca1, sa1, 1)
                emit_chain_block(xt1, w3e1, sa1, 3)
                emit_mul_group(1, xt1, ot1, ca1, sa1, 2)
                emit_mul_group(1, xt1, ot1, ca1, sa1, 3)

    nc.finalize()
    return nc


def _get_nc(n_iter=1):
    key = ("nc", n_iter)
    if key not in _CACHE:
        _CACHE[key] = _build(n_iter)
    return _CACHE[key]


def _make_in_maps(inputs):
    x = np.ascontiguousarray(
        np.asarray(inputs["x"], dtype=np.float32).astype(np.float16)
    )
    w1 = np.asarray(inputs["w1"], dtype=np.float32)
    b1 = np.asarray(inputs["b1"], dtype=np.float32)
    w2 = np.asarray(inputs["w2"], dtype=np.float32)
    b2 = np.asarray(inputs["b2"], dtype=np.float32)
    w3 = np.asarray(inputs["w3"], dtype=np.float32)
    b3 = np.asarray(inputs["b3"], dtype=np.float32)
    bn_gamma = np.asarray(inputs["bn_gamma"], dtype=np.float32)
    bn_beta = np.asarray(inputs["bn_beta"], dtype=np.float32)
    bn_mean = np.asarray(inputs["bn_mean"], dtype=np.float32)
    bn_var = np.asarray(inputs["bn_var"], dtype=np.float32)
    w4 = np.asarray(inputs["w4"], dtype=np.float32)
    b4 = np.asarray(inputs["b4"], dtype=np.float32)

    # ---- host-side weight folding into blobs (tiny) ----
    inv = bn_gamma / np.sqrt(bn_var + BN_EPS)                   # [CR]
    w1nT = (w1.T / float(N)).reshape(CCH, P, CR).transpose(1, 0, 2)
    w3Ti = (w3.T * inv[None, :]).reshape(CCH, P, CR).transpose(1, 0, 2)
    b3e = b3 * inv + bn_beta - bn_mean * inv

    wbh = np.zeros((P, HBLOB), np.float16)
    wbh[:, _W3 : _W3 + 512] = w3Ti.reshape(P, 512).astype(np.float16)
    wbh[:, _W1 : _W1 + 512] = w1nT.reshape(P, 512).astype(np.float16)
    wbh[:, _W2 : _W2 + 512] = w2.T.astype(np.float16)            # [CR->P, C]
    wbh[:, _W4 : _W4 + P] = np.repeat(
        w4.reshape(CR, 1).astype(np.float16), P, axis=1
    )
    wbf = np.zeros((P, FBLOB), np.float32)
    wbf[:, _B1] = b1
    wbf[:, _B3] = b3e
    wbf[:, _B2C : _B2C + CCH] = b2.reshape(CCH, P).T
    wbf[:, _B4] = b4[0]

    in_maps = []
    for i in range(NCORES):
        in_maps.append(
            {
                "xs": x[i * BPC : (i + 1) * BPC].reshape(BPC * C, N),
                "wblobh": wbh,
                "wblobf": wbf,
            }
        )
    return in_maps


def kernel(**inputs):
    nc = _get_nc()
    in_maps = _make_in_maps(inputs)

    from concourse.bass_utils import run_bass_kernel_spmd

    res = run_bass_kernel_spmd(nc, in_maps, core_ids=list(range(NCORES)))
    _CACHE["last_result"] = res
    out = np.concatenate(
        [
            res.results[i]["outv"].astype(np.float32).reshape(BPC, C, N)
            for i in range(NCORES)
        ],
        axis=0,
    )
    return out


# revision 40
# speedup vs baseline: 1.2267x; 1.2267x over previous
"""EnhancedAttentionModule Trainium2 kernel.

x: [16, 512, 4096] f32.  Module:
    pooled = mean_n(x)                      # [B, C]
    h  = relu(pooled @ w1.T + b1)           # [B, C/4]
    ca = sigmoid(h @ w2.T + b2)             # [B, C]  (channel attention)
    x_ca = x * ca[:, :, None]
    h2 = BN(w3 @ x_ca + b3); h2 = relu(h2)  # [B, C/4, N]
    sa = sigmoid(w4 @ h2 + b4)              # [B, 1, N] (spatial attention)
    out = x + x_ca * sa = x * (1 + ca*sa)

Restructuring:
  - The problem is HBM-DMA bound: all DMA serializes on one shared
    engine pool at ~360 GB/s. x (and out) are stored in DRAM as fp16
    (host converts); accumulation stays f32 in PSUM. Measured
    end-to-end rel err ~1e-3 (gate 2e-2).
  - mean divisor folded into w1, BN folded into w3/bias (host); all
    matmul weights shipped fp16 in one blob (Matmult forbids mixing
    16/32-bit inputs; fp16 runs 1 cycle/row on PE).
  - ca folded into the w3 matmul weights on device (w3e = w3Ti * ca).
  - sa is produced REPLICATED across all 128 partitions for free: the
    w4 matmul uses a [CR, 128] all-equal-columns lhsT, so the sigmoid
    (cost = free size) directly yields [128, N] fp16 sa.
  - out = x * (1 + ca[c]*sa[n]): s2 = sa*ca_j + 1 via DVE tensor_scalar
    (4x fp16 mode); the multiplies are split DVE (2x fp16 mode) / Pool
    (gpsimd tensor_tensor) per 1024-block so neither engine's in-order
    queue becomes the tail.
  - pooled sums all run on DVE (in-place x*1.0 with accum_out, 4x
    mode); batch-1's are interleaved between batch-0's multiply groups
    so the in-order DVE queue never head-blocks on a not-yet-loaded
    tile.
  - stores go out in 1024-wide quarter-tiles immediately after each
    block's multiply, so the serial DMA queue never waits for a full
    tile; loads for both batches are issued up front.
  - a t~0 dummy sigmoid pins the one ACT table set that covers
    Copy/Relu/Sigmoid (no mid-chain 1.3us table switches); tiny dummy
    matmuls tied to each x-tile load keep the PE p-state ramped.

Sharding: data-parallel over batch. 8 cores x 2 batches each. Weights
replicated. No collectives. Per core: 8.4 MB HBM read + 8.4 MB write
plus ~0.5 MB weights - the serial-DMA roofline for this problem.
"""

import numpy as np

B, C, N = 16, 512, 4096
CR = C // 4  # 128
P = 128      # partitions
NCORES = 8
BPC = B // NCORES        # batches per core = 2
CCH = C // P             # channel chunks per batch = 4
NK = N // 1024           # 1024-wide chain blocks = 4
BN_EPS = 1e-5

# fp16 weight blob ([128, HBLOB])
_W3 = 0          # w3Ti as [p, j, m]: cols [0, 512)
_W1 = 512        # w1nT as [p, j, m]: cols [512, 1024)
_W2 = 1024       # w2T: cols [1024, 1536)
_W4 = 1536       # w4 replicated into 128 cols: [1536, 1664)
HBLOB = 1664
# f32 small blob ([128, FBLOB]): biases
_B1 = 0
_B3 = 1
_B2C = 2         # cols [2, 6)
_B4 = 6          # replicated down all 128 rows
FBLOB = 7

_CACHE = {}


def _build(n_iter=1):
    import concourse.bacc as bacc
    import concourse.tile as tile
    from concourse import mybir

    f32 = mybir.dt.float32
    f16 = mybir.dt.float16
    AF = mybir.ActivationFunctionType
    ALU = mybir.AluOpType

    nc = bacc.Bacc(None)

    xs = nc.dram_tensor("xs", [BPC * C, N], f16, kind="ExternalInput")
    out = nc.dram_tensor("outv", [BPC * C, N], f16, kind="ExternalOutput")
    wbh_d = nc.dram_tensor("wblobh", [P, HBLOB], f16, kind="ExternalInput")
    wbf_d = nc.dram_tensor("wblobf", [P, FBLOB], f32, kind="ExternalInput")

    xs_t = xs.rearrange("(t p) n -> t p n", p=P)      # 8 tiles [128, 4096]
    out_t = out.rearrange("(t p) n -> t p n", p=P)

    with tile.TileContext(nc) as tc:
        with (
            tc.tile_pool(name="wpool", bufs=1) as wpool,
            tc.tile_pool(name="xpool", bufs=BPC * CCH) as xpool,
            tc.tile_pool(name="opool", bufs=BPC * CCH * NK) as opool,
            tc.tile_pool(name="small", bufs=6) as small,
            tc.tile_pool(name="wefpool", bufs=2 * CCH) as wefpool,
            tc.tile_pool(name="h2spool", bufs=2) as h2spool,
            tc.tile_pool(name="sapool", bufs=2) as sapool,
            tc.tile_pool(name="s2pool", bufs=12) as s2pool,
            tc.tile_pool(name="ps_hca", bufs=1, space="PSUM") as ps_hca,
            tc.tile_pool(name="ps_h2", bufs=2, space="PSUM") as ps_h2,
            tc.tile_pool(name="ps_sa", bufs=1, space="PSUM") as ps_sa,
            tc.tile_pool(name="ps_junk", bufs=1, space="PSUM") as ps_junk,
        ):
            wbh = wpool.tile([P, HBLOB], f16)
            wbf = wpool.tile([P, FBLOB], f32)
            w3Ti_sb = wbh[:, _W3 : _W3 + 512].rearrange("p (j m) -> p j m", j=CCH)
            w1nT_sb = wbh[:, _W1 : _W1 + 512].rearrange("p (j m) -> p j m", j=CCH)
            w2T_sb = wbh[:, _W2 : _W2 + 512]
            w4r_sb = wbh[:, _W4 : _W4 + P]
            b1_sb = wbf[:, _B1 : _B1 + 1]
            b3e_sb = wbf[:, _B3 : _B3 + 1]
            b2c_sb = wbf[:, _B2C : _B2C + CCH]
            b4_sb = wbf[:, _B4 : _B4 + 1]

            # dummy tiles: pin the sigmoid act table at t~0 (the
            # sigmoid_and_others set also serves Copy and Relu, so no
            # further table loads occur) and seed the PE p-state ramp.
            junk = wpool.tile([P, 2], f16)
            junkf = wpool.tile([1, 2], f32)
            psj = ps_junk.tile([P, 2], f32)
            nc.vector.memset(junk, 1.0)
            nc.scalar.activation(junkf, junk[0:1, :], AF.Sigmoid)
            nc.tensor.matmul(psj[0:1, :], lhsT=junk[:, 0:1], rhs=junk, start=True, stop=True)

            def pe_warm(t):
                # tiny matmul tied to a fresh x tile: keeps the PE busy
                # streak alive through the load phase so the real h2
                # matmuls run at the full 2.4 GHz p-state.
                nc.tensor.matmul(
                    psj[0:1, 0:1], lhsT=t[:, 0:1], rhs=t[:, 1:2],
                    start=True, stop=True,
                )

            def emit_weight_dmas():
                nc.sync.dma_start(out=wbh, in_=wbh_d[:, :])
                nc.sync.dma_start(out=wbf, in_=wbf_d[:, :])

            for _it in range(n_iter):
                # ---- all x loads emitted up front (both batches) so the
                # serial DMA resource runs them back-to-back.
                xts = []
                for b in range(BPC):
                    xt = []
                    for j in range(CCH):
                        t = xpool.tile([P, N], f16, tag="xt")
                        xt.append(t)
                        if j == 3:
                            # last tile of the batch: split load so the
                            # critical pooled reduction starts on the
                            # first half while the second streams in.
                            nc.sync.dma_start(
                                out=t[:, 0:2048], in_=xs_t[b * CCH + j][:, 0:2048]
                            )
                            nc.sync.dma_start(
                                out=t[:, 2048:4096],
                                in_=xs_t[b * CCH + j][:, 2048:4096],
                            )
                        else:
                            nc.sync.dma_start(out=t, in_=xs_t[b * CCH + j])
                        pe_warm(t)
                    xts.append(xt)
                    if b == 0 and _it == 0:
                        emit_weight_dmas()

                def emit_pooled_halves(t, pooled, j):
                    # two half-tile reductions (the halves arrive as
                    # separate DMAs); the MLP matmul accumulates the
                    # partials directly - it is linear in pooled.
                    for h in range(2):
                        pj = small.tile([P, 1], f32, tag="pooled")
                        nc.vector.tensor_scalar(
                            t[:, h * 2048 : (h + 1) * 2048],
                            t[:, h * 2048 : (h + 1) * 2048],
                            1.0, 0.0, ALU.mult, ALU.add, accum_out=pj,
                        )
                        ph = small.tile([P, 1], f16, tag="pooledh")
                        nc.gpsimd.tensor_copy(ph, pj)
                        pooled.append((j, ph))
                    pe_warm(t)

                def emit_pooled(t, act=False):
                    # in-place identity with free-dim accumulator: ACT
                    # (copy) for tiles arriving while ACT idles, DVE (4x
                    # fp16 tensor_scalar) for critical late tiles. The
                    # tiny f32->fp16 copy for the fp16 MLP matmul runs on
                    # Pool to keep the DVE queue clear.
                    pj = small.tile([P, 1], f32, tag="pooled")
                    if act:
                        nc.scalar.activation(t, t, AF.Copy, accum_out=pj)
                    else:
                        nc.vector.tensor_scalar(
                            t, t, 1.0, 0.0, ALU.mult, ALU.add, accum_out=pj
                        )
                    ph = small.tile([P, 1], f16, tag="pooledh")
                    nc.gpsimd.tensor_copy(ph, pj)
                    pe_warm(t)
                    return ph

                def emit_mlp(pooled):
                    # channel attention MLP (all-fp16 matmuls); returns
                    # ca as per-partition columns [P, CCH] f32 (scalar
                    # ptr operands must be f32). `pooled` is a list of
                    # (j, partial-sum) pairs: a tile's pooled sum may
                    # arrive as several partials (the matmul accumulates
                    # them - it's linear).
                    psum_hca = ps_hca.tile([P, 8], f32, tag="hca")
                    psum_h = psum_hca[:, 0:1]
                    psum_ca = psum_hca[:, 4:8]
                    for i, (j, ph) in enumerate(pooled):
                        nc.tensor.matmul(
                            psum_h,
                            lhsT=w1nT_sb[:, j, :],
                            rhs=ph,
                            start=(i == 0),
                            stop=(i == len(pooled) - 1),
                        )
                    h_sb = small.tile([P, 1], f16, tag="h")
                    nc.scalar.activation(h_sb, psum_h, AF.Relu, bias=b1_sb)
                    for j in range(CCH):
                        nc.tensor.matmul(
                            psum_ca[:, j : j + 1],
                            lhsT=w2T_sb[:, j * P : (j + 1) * P],
                            rhs=h_sb,
                            start=True,
                            stop=True,
                        )
                    ca_sb = small.tile([P, CCH], f32, tag="ca")
                    for j in range(CCH):
                        nc.scalar.activation(
                            ca_sb[:, j : j + 1],
                            psum_ca[:, j : j + 1],
                            AF.Sigmoid,
                            bias=b2c_sb[:, j : j + 1],
                        )
                    # fold ca into w3 (fp16 weights for the fp16 h2 matmul)
                    w3e = []
                    for j in range(CCH):
                        we = wefpool.tile([P, CR], f16, tag="w3e")
                        nc.vector.tensor_scalar_mul(
                            we, w3Ti_sb[:, j, :], ca_sb[:, j : j + 1]
                        )
                        w3e.append(we)
                    pe_warm(w3e[0])
                    return ca_sb, w3e

                def emit_chain_block(xt, w3e, sa_sb, k):
                    # h2 = relu(w3e @ x + b3e); sa = sigmoid(w4r @ h2 + b4)
                    # on one 1024-wide block, sa replicated on all rows.
                    # matmul outputs are 512-wide (a PSUM bank holds 512
                    # f32); the ACT ops span both banks in one 1024-wide
                    # instruction (PSUM-crossing APs are legal for ACT).
                    lo = k * 1024
                    psum_h2 = ps_h2.tile([P, 1024], f32, tag="ph2")
                    for hh in range(2):
                        o = lo + hh * 512
                        for j in range(CCH):
                            nc.tensor.matmul(
                                psum_h2[:, hh * 512 : (hh + 1) * 512],
                                lhsT=w3e[j],
                                rhs=xt[j][:, o : o + 512],
                                start=(j == 0),
                                stop=(j == CCH - 1),
                            )
                    h2s = h2spool.tile([P, 1024], f16, tag="h2s")
                    nc.scalar.activation(h2s, psum_h2, AF.Relu, bias=b3e_sb)
                    psum_sa = ps_sa.tile([P, 1024], f32, tag="psa")
                    for hh in range(2):
                        nc.tensor.matmul(
                            psum_sa[:, hh * 512 : (hh + 1) * 512],
                            lhsT=w4r_sb,
                            rhs=h2s[:, hh * 512 : (hh + 1) * 512],
                            start=True,
                            stop=True,
                        )
                    nc.scalar.activation(
                        sa_sb[:, lo : lo + 1024], psum_sa, AF.Sigmoid, bias=b4_sb
                    )

                def emit_mul_group(b, xt, ot, ca_sb, sa_sb, k, s2_act=False, js=None):
                    # out = x * (1 + ca_j*sa) for 1024-block k, all 4 j.
                    # s2 on DVE (tensor_scalar, 4x); multiplies j0-j2 on
                    # DVE (2x), j3 on Pool (gpsimd) so neither in-order
                    # queue becomes the tail. Multiplies write SEPARATE
                    # output tiles: writing the x tile in place makes
                    # every mul wait (whole-tile WAR) for the tile's last
                    # h2-matmul read, which costs ~8us per batch. Each
                    # half-tile store goes out immediately after that
                    # tile's mul in an odd group (its 2 blocks are done).
                    lo = k * 1024
                    for j in js if js is not None else (0, 3, 1, 2):
                        s2 = s2pool.tile([P, 1024], f16, tag="s2")
                        if s2_act:
                            # ACT is idle after its sigmoid chains; its
                            # func(in*scale+bias) form computes sa*ca+1
                            # directly, relieving the DVE tail.
                            nc.scalar.activation(
                                s2,
                                sa_sb[:, lo : lo + 1024],
                                AF.Copy,
                                bias=1.0,
                                scale=ca_sb[:, j : j + 1],
                            )
                        else:
                            nc.vector.tensor_scalar(
                                s2,
                                sa_sb[:, lo : lo + 1024],
                                ca_sb[:, j : j + 1],
                                1.0,
                                ALU.mult,
                                ALU.add,
                            )
                        # Pool helps early/mid groups only: its 2.13us/mul
                        # pace must never gate the batch's last stores.
                        on_pool = (j == 3 and (k < 3 or b == 0)) or (j == 2 and b == 0 and k < 3)
                        eng = nc.gpsimd if on_pool else nc.vector
                        eng.tensor_mul(
                            ot[j][k],
                            xt[j][:, lo : lo + 1024],
                            s2,
                        )
                        nc.sync.dma_start(
                            out=out_t[b * CCH + j][:, lo : lo + 1024],
                            in_=ot[j][k],
                        )

                # ---------- batch 0 ----------
                xt0, xt1 = xts
                # per-quarter output tiles: a shared per-tile output
                # buffer makes each store (reader) gate the next group's
                # mul (writer) via whole-tile WAR + sem prop. Dedicated
                # [P, 1024] tiles kill that false dependency.
                ot0 = [
                    [
                        opool.tile([P, 1024], f16, tag="ot", name=f"ot0_{j}_{k}_{_it}")
                        for k in range(NK)
                    ]
                    for j in range(CCH)
                ]
                ot1 = [
                    [
                        opool.tile([P, 1024], f16, tag="ot", name=f"ot1_{j}_{k}_{_it}")
                        for k in range(NK)
                    ]
                    for j in range(CCH)
                ]
                pooled0 = [(j, emit_pooled(xt0[j], act=(j < 2))) for j in range(3)]
                emit_pooled_halves(xt0[3], pooled0, 3)
                ca0, w3e0 = emit_mlp(pooled0)
                sa0 = sapool.tile([P, N], f16, tag="sa")
                pooled1 = []

                # chain blocks + mul groups pipelined; batch-1 pooled
                # interleaved into the DVE stream as its tiles land.
                emit_chain_block(xt0, w3e0, sa0, 0)
                pooled1.append((0, emit_pooled(xt1[0])))  # b1 t0 (early)
                emit_chain_block(xt0, w3e0, sa0, 1)
                emit_mul_group(0, xt0, ot0, ca0, sa0, 0)
                pooled1.append((1, emit_pooled(xt1[1])))
                emit_chain_block(xt0, w3e0, sa0, 2)
                emit_mul_group(0, xt0, ot0, ca0, sa0, 1)
                pooled1.append((2, emit_pooled(xt1[2])))
                emit_chain_block(xt0, w3e0, sa0, 3)
                emit_mul_group(0, xt0, ot0, ca0, sa0, 2, js=(0, 3))
                emit_pooled_halves(xt1[3], pooled1, 3)
                emit_mul_group(0, xt0, ot0, ca0, sa0, 2, js=(1, 2))

                # ---------- batch 1 (MLP emitted before batch 0's last
                # mul group so its DVE folds aren't queued behind it) ----
                ca1, w3e1 = emit_mlp(pooled1)
                sa1 = sapool.tile([P, N], f16, tag="sa")
                emit_mul_group(0, xt0, ot0, ca0, sa0, 3)
                emit_chain_block(xt1, w3e1, sa1, 0)
                emit_chain_block(xt1, w3e1, sa1, 1)
                emit_mul_group(1, xt1, ot1, ca1, sa1, 0)
                emit_chain_block(xt1, w3e1, sa1, 2)
                emit_mul_group(1, xt1, ot1, ca1, sa1, 1)
                emit_chain_block(xt1, w3e1, sa1, 3)
                emit_mul_group(1, xt1, ot1, ca1, sa1, 2)
                emit_mul_group(1, xt1, ot1, ca1, sa1, 3)

    nc.finalize()
    return nc


def _get_nc(n_iter=1):
    key = ("nc", n_iter)
    if key not in _CACHE:
        _CACHE[key] = _build(n_iter)
    return _CACHE[key]


def _make_in_maps(inputs):
    x = np.ascontiguousarray(
        np.asarray(inputs["x"], dtype=np.float32).astype(np.float16)
    )
    w1 = np.asarray(inputs["w1"], dtype=np.float32)
    b1 = np.asarray(inputs["b1"], dtype=np.float32)
    w2 = np.asarray(inputs["w2"], dtype=np.float32)
    b2 = np.asarray(inputs["b2"], dtype=np.float32)
    w3 = np.asarray(inputs["w3"], dtype=np.float32)
    b3 = np.asarray(inputs["b3"], dtype=np.float32)
    bn_gamma = np.asarray(inputs["bn_gamma"], dtype=np.float32)
    bn_beta = np.asarray(inputs["bn_beta"], dtype=np.float32)
    bn_mean = np.asarray(inputs["bn_mean"], dtype=np.float32)
    bn_var = np.asarray(inputs["bn_var"], dtype=np.float32)
    w4 = np.asarray(inputs["w4"], dtype=np.float32)
    b4 = np.asarray(inputs["b4"], dtype=np.float32)

    # ---- host-side weight folding into blobs (tiny) ----
    inv = bn_gamma / np.sqrt(bn_var + BN_EPS)                   # [CR]
    w1nT = (w1.T / float(N)).reshape(CCH, P, CR).transpose(1, 0, 2)
    w3Ti = (w3.T * inv[None, :]).reshape(CCH, P, CR).transpose(1, 0, 2)
    b3e = b3 * inv + bn_beta - bn_mean * inv

    wbh = np.zeros((P, HBLOB), np.float16)
    wbh[:, _W3 : _W3 + 512] = w3Ti.reshape(P, 512).astype(np.float16)
    wbh[:, _W1 : _W1 + 512] = w1nT.reshape(P, 512).astype(np.float16)
    wbh[:, _W2 : _W2 + 512] = w2.T.astype(np.float16)            # [CR->P, C]
    wbh[:, _W4 : _W4 + P] = np.repeat(
        w4.reshape(CR, 1).astype(np.float16), P, axis=1
    )
    wbf = np.zeros((P, FBLOB), np.float32)
    wbf[:, _B1] = b1
    wbf[:, _B3] = b3e
    wbf[:, _B2C : _B2C + CCH] = b2.reshape(CCH, P).T
    wbf[:, _B4] = b4[0]

    in_maps = []
    for i in range(NCORES):
        in_maps.append(
            {
                "xs": x[i * BPC : (i + 1) * BPC].reshape(BPC * C, N),
                "wblobh": wbh,
                "wblobf": wbf,
            }
        )
    return in_maps


def kernel(**inputs):
    nc = _get_nc()
    in_maps = _make_in_maps(inputs)

    from concourse.bass_utils import run_bass_kernel_spmd

    res = run_bass_kernel_spmd(nc, in_maps, core_ids=list(range(NCORES)))
    _CACHE["last_result"] = res
    out = np.concatenate(
        [
            res.results[i]["outv"].astype(np.float32).reshape(BPC, C, N)
            for i in range(NCORES)
        ],
        axis=0,
    )
    return out
